# revision 1
# baseline (speedup 1.0000x reference)
"""MQA attention with ALiBi + causal mask on 8 TRN2 NeuronCores.

Problem: hidden_states [2,2048,4096] @ Wq -> 32 query heads of 128; single
KV head via Wkv; scores + ALiBi bias + causal mask; softmax; @ Wo.

Distribution (differs from plain head-TP to avoid the 64 MiB AllReduce):
- Tokens are flattened batch-major: t = b*2048 + s in [0, 4096). Core c owns
  tokens [512c, 512(c+1)) for all projections (q/k/v and the output
  projection). Output rows are disjoint -> host just concatenates.
- Attention itself is head-sharded (core h handles heads 4h..4h+3 for ALL
  tokens), which makes every core's causal workload identical. Data moves
  between the two shardings with two small AllToAlls of qT / attnT (4 MiB
  per core each, bf16) plus AllGathers of the tiny single-head K/V.
- Everything streams through the TensorEngine in bf16 (rel-err budget 2e-2);
  softmax runs in f32 without max-subtraction (scores are O(10), ALiBi bias
  is <= 0, so exp never overflows and the diagonal keeps denominators O(1)).

Softmax layout trick: scores are built transposed (scoresT[kpos, q]) so the
probs @ V matmul needs no transposes; the denominator comes from a
ones-stationary matmul accumulated alongside, and the per-q reciprocal is
broadcast across partitions with a K=1 outer-product matmul.
"""
import math
import os

import numpy as np
import ml_dtypes

import concourse.bass as bass
from concourse import bacc
import concourse.mybir as mybir
from concourse.tile import TileContext
from concourse.bass_utils import run_bass_kernel_spmd

B, S, H, NH, HD = 2, 2048, 4096, 32, 128
NC = 8              # cores
TPC = 512           # tokens per core
KC = H // 128       # 32 contraction chunks
GQ = 8              # 256-token q blocks per batch
NREL = GQ * (GQ + 1)  # 72 (g, j) rel tiles
SCALE = HD ** -0.5
bf16 = mybir.dt.bfloat16
f32 = mybir.dt.float32
Exp = mybir.ActivationFunctionType.Exp
Copy = mybir.ActivationFunctionType.Copy
MULT = mybir.AluOpType.mult
ADD = mybir.AluOpType.add

_CACHE = {}
LAST_EXEC_NS = None


def _alibi_slopes(n_heads):
    closest_pow2 = 2 ** math.floor(math.log2(n_heads))
    base = 2.0 ** (-(2.0 ** -(math.log2(closest_pow2) - 3)))
    slopes = [base ** i for i in range(1, closest_pow2 + 1)]
    if closest_pow2 != n_heads:
        extra_base = 2.0 ** (-(2.0 ** -(math.log2(2 * closest_pow2) - 3)))
        n_extra = min(closest_pow2, n_heads - closest_pow2)
        slopes += [extra_base ** i for i in range(1, 2 * n_extra + 1, 2)]
    return np.asarray(slopes, dtype=np.float32)


def _rel_idx(g, j):
    return g * g + g + j


def _build_rel():
    rel = np.empty((128, NREL * 256), np.float32)
    for g in range(GQ):
        for j in range(2 * g + 2):
            kpos = 128 * j + np.arange(128)[:, None]
            q = 256 * g + np.arange(256)[None, :]
            r = (kpos - q).astype(np.float32)
            r[kpos > q] = -30000.0
            i = _rel_idx(g, j)
            rel[:, 256 * i:256 * (i + 1)] = r
    return rel.astype(ml_dtypes.bfloat16)


def _build_nc():
    nc = bacc.Bacc(num_devices=NC)
    hsT = nc.declare_dram_parameter("hsT", [H, TPC], bf16, isOutput=False)
    Wq_t = nc.declare_dram_parameter("Wq_t", [32, H, 128], bf16, isOutput=False)
    Wkv = nc.declare_dram_parameter("Wkv", [H, 256], bf16, isOutput=False)
    Wo_t = nc.declare_dram_parameter("Wo_t", [8, H, 512], bf16, isOutput=False)
    rel = nc.declare_dram_parameter("rel", [128, NREL * 256], bf16, isOutput=False)
    slopes = nc.declare_dram_parameter("slopes", [128, 4], f32, isOutput=False)
    out = nc.declare_dram_parameter("out", [TPC, H], f32, isOutput=True)

    with TileContext(nc) as tc:
        with (
            tc.tile_pool(name="dram", bufs=1, space="DRAM") as dram,
            tc.tile_pool(name="const", bufs=1) as const,
            tc.tile_pool(name="psum", bufs=1, space="PSUM") as psum,
        ):
            kT_in = dram.tile([128, TPC], bf16)
            kT_ag = dram.tile([128 * NC, TPC], bf16, addr_space="Shared")
            v_in = dram.tile([TPC, 128], bf16)
            v_ag = dram.tile([TPC * NC, 128], bf16, addr_space="Shared")
            q_in = dram.tile([H, TPC], bf16)
            q_a2a = dram.tile([H, TPC], bf16)
            a_in = dram.tile([H, TPC], bf16)
            a_a2a = dram.tile([H, TPC], bf16)

            rel_sb = const.tile([128, NREL * 256], bf16)
            nc.sync.dma_start(out=rel_sb[:], in_=rel[:])
            slopes_sb = const.tile([128, 4], f32)
            nc.sync.dma_start(out=slopes_sb[:], in_=slopes[:])
            ones_col = const.tile([128, 1], bf16)
            nc.vector.memset(ones_col[:], 1.0)
            ones_row = const.tile([1, 128], bf16)
            nc.vector.memset(ones_row[:], 1.0)

            # ---------------- Phase 1: q/k/v projections on my 512 tokens ---
            with tc.tile_pool(name="ph1", bufs=1) as ph1:
                hsT_sb = ph1.tile([128, KC, TPC], bf16)
                nc.sync.dma_start(
                    out=hsT_sb[:],
                    in_=hsT.rearrange("(k p) t -> p k t", p=128))
                Wkv_sb = ph1.tile([128, KC, 256], bf16)
                nc.sync.dma_start(
                    out=Wkv_sb[:],
                    in_=Wkv.rearrange("(k p) c -> p k c", p=128))

                kT_ps = psum.tile([128, TPC], f32, tag="str", bufs=3)
                for k in range(KC):
                    nc.tensor.matmul(kT_ps[:], lhsT=Wkv_sb[:, k, 0:128],
                                     rhs=hsT_sb[:, k, :],
                                     start=(k == 0), stop=(k == KC - 1))
                kT_sb = ph1.tile([128, TPC], bf16)
                nc.scalar.activation(kT_sb[:], kT_ps[:], Copy)
                nc.sync.dma_start(out=kT_in[:], in_=kT_sb[:])

                for t4 in range(4):
                    v_ps = psum.tile([128, 128], f32, tag="str", bufs=3,
                                     name="v_ps")
                    for k in range(KC):
                        nc.tensor.matmul(
                            v_ps[:], lhsT=hsT_sb[:, k, 128 * t4:128 * (t4 + 1)],
                            rhs=Wkv_sb[:, k, 128:256],
                            start=(k == 0), stop=(k == KC - 1))
                    v_sb = ph1.tile([128, 128], bf16, tag="v_sb", bufs=3,
                                    name="v_sb")
                    nc.scalar.activation(v_sb[:], v_ps[:], Copy)
                    nc.sync.dma_start(out=v_in[128 * t4:128 * (t4 + 1), :],
                                      in_=v_sb[:])

                for dq in range(32):
                    wq_sb = ph1.tile([128, KC, 128], bf16, tag="wq", bufs=3,
                                     name="wq_sb")
                    nc.sync.dma_start(
                        out=wq_sb[:],
                        in_=Wq_t[dq].rearrange("(k p) m -> p k m", p=128))
                    q_ps = psum.tile([128, TPC], f32, tag="acc", bufs=4,
                                     name="q_ps")
                    for k in range(KC):
                        nc.tensor.matmul(q_ps[:], lhsT=wq_sb[:, k, :],
                                         rhs=hsT_sb[:, k, :],
                                         start=(k == 0), stop=(k == KC - 1))
                    q_sb = ph1.tile([128, TPC], bf16, tag="qstage", bufs=3,
                                    name="q_sb")
                    nc.scalar.activation(q_sb[:], q_ps[:], Copy, scale=SCALE)
                    nc.sync.dma_start(out=q_in[128 * dq:128 * (dq + 1), :],
                                      in_=q_sb[:])

            # ---------------- Phase 2: collectives ------------------------
            grp = [list(range(NC))]
            nc.gpsimd.collective_compute(
                "AllGather", mybir.AluOpType.bypass, replica_groups=grp,
                ins=[kT_in[:]], outs=[kT_ag[:]])
            nc.gpsimd.collective_compute(
                "AllGather", mybir.AluOpType.bypass, replica_groups=grp,
                ins=[v_in[:]], outs=[v_ag[:]])
            nc.gpsimd.collective_compute(
                "AllToAll", mybir.AluOpType.bypass, replica_groups=grp,
                ins=[q_in[:]], outs=[q_a2a[:]])

            # ---------------- Phase 3: attention for my 4 heads ------------
            with tc.tile_pool(name="attn", bufs=1) as attn:
                qT = {}
                aT = {}
                for b in range(B):
                    for h in range(4):
                        t = attn.tile([128, 4, TPC], bf16, name=f"qT_{b}_{h}")
                        nc.sync.dma_start(
                            out=t[:],
                            in_=q_a2a.rearrange("(j h p) t -> h p j t",
                                                h=4, p=128)[h][:, 4 * b:4 * b + 4, :])
                        qT[b, h] = t
                        aT[b, h] = attn.tile([128, 4, TPC], bf16,
                                             name=f"aT_{b}_{h}")
                kT_b = {}
                v_b = {}
                for b in range(B):
                    t = attn.tile([128, 4, TPC], bf16, name=f"kT_{b}")
                    nc.sync.dma_start(
                        out=t[:],
                        in_=kT_ag.rearrange("(s p) t -> p s t",
                                            p=128)[:, 4 * b:4 * b + 4, :])
                    kT_b[b] = t
                    t = attn.tile([128, 16, 128], bf16, name=f"v_{b}")
                    nc.sync.dma_start(
                        out=t[:],
                        in_=v_ag.rearrange("(b j p) d -> b p j d",
                                           b=2, p=128)[b])
                    v_b[b] = t

                for b in range(B):
                    for hp in range(2):
                        heads = (2 * hp, 2 * hp + 1)
                        for g in range(GQ):
                            nch = 2 * (g + 1)
                            at = [psum.tile([128, 256], f32, tag="acc", bufs=4,
                                            name=f"at{hi}")
                                  for hi in range(2)]
                            den = psum.tile([1, 512], f32, tag="den", bufs=1,
                                            name="den")
                            for j in range(nch):
                                expj = attn.tile([128, 512], bf16, tag="exp",
                                                 bufs=4, name="expj")
                                for hi, h in enumerate(heads):
                                    s_ps = psum.tile([128, 256], f32, tag="str",
                                                     bufs=3, name="s_ps")
                                    nc.tensor.matmul(
                                        s_ps[:],
                                        lhsT=kT_b[b][:, j // 4,
                                                     128 * (j % 4):128 * (j % 4 + 1)],
                                        rhs=qT[b, h][:, g // 2,
                                                     256 * (g % 2):256 * (g % 2 + 1)],
                                        start=True, stop=True)
                                    tmp = attn.tile([128, 256], f32, tag="stt",
                                                    bufs=3, name="tmp")
                                    i0 = 256 * _rel_idx(g, j)
                                    nc.vector.scalar_tensor_tensor(
                                        out=tmp[:], in0=rel_sb[:, i0:i0 + 256],
                                        scalar=slopes_sb[:, h:h + 1],
                                        in1=s_ps[:], op0=MULT, op1=ADD)
                                    nc.scalar.activation(
                                        expj[:, 256 * hi:256 * (hi + 1)],
                                        tmp[:], Exp)
                                for hi in range(2):
                                    nc.tensor.matmul(
                                        at[hi][:],
                                        lhsT=v_b[b][:, j, :],
                                        rhs=expj[:, 256 * hi:256 * (hi + 1)],
                                        start=(j == 0), stop=(j == nch - 1))
                                nc.tensor.matmul(
                                    den[:], lhsT=ones_col[:], rhs=expj[:],
                                    start=(j == 0), stop=(j == nch - 1))
                            den_sb = attn.tile([1, 512], f32, tag="den_sb",
                                               bufs=2, name="den_sb")
                            nc.vector.tensor_copy(out=den_sb[:], in_=den[:])
                            rec = attn.tile([1, 512], f32, tag="rec", bufs=2,
                                            name="rec")
                            nc.vector.reciprocal_approx_fast(out=rec[:],
                                                             in_=den_sb[:])
                            rec_bf = attn.tile([1, 512], bf16, tag="rec_bf",
                                               bufs=2, name="rec_bf")
                            nc.vector.tensor_copy(out=rec_bf[:], in_=rec[:])
                            rb_ps = psum.tile([128, 512], f32, tag="str",
                                              bufs=3, name="rb_ps")
                            nc.tensor.matmul(rb_ps[:], lhsT=ones_row[:],
                                             rhs=rec_bf[:], start=True,
                                             stop=True)
                            rb_sb = attn.tile([128, 512], f32, tag="rb",
                                              bufs=2, name="rb_sb")
                            nc.scalar.activation(rb_sb[:], rb_ps[:], Copy)
                            for hi, h in enumerate(heads):
                                nc.vector.tensor_tensor(
                                    out=aT[b, h][:, g // 2,
                                                 256 * (g % 2):256 * (g % 2 + 1)],
                                    in0=at[hi][:],
                                    in1=rb_sb[:, 256 * hi:256 * (hi + 1)],
                                    op=MULT)
                    for h in range(4):
                        nc.sync.dma_start(
                            out=a_in.rearrange("(j h p) t -> h p j t",
                                               h=4, p=128)[h][:, 4 * b:4 * b + 4, :],
                            in_=aT[b, h][:])

            nc.gpsimd.collective_compute(
                "AllToAll", mybir.AluOpType.bypass, replica_groups=grp,
                ins=[a_in[:]], outs=[a_a2a[:]])

            # ---------------- Phase 4: output projection -------------------
            with tc.tile_pool(name="ph4", bufs=1) as ph4:
                att_sb = ph4.tile([128, KC, TPC], bf16)
                nc.sync.dma_start(
                    out=att_sb[:],
                    in_=a_a2a.rearrange("(k p) t -> p k t", p=128))
                for n8 in range(8):
                    wo_sb = ph4.tile([128, KC, 512], bf16, tag="wo", bufs=2,
                                     name="wo_sb")
                    nc.sync.dma_start(
                        out=wo_sb[:],
                        in_=Wo_t[n8].rearrange("(k p) n -> p k n", p=128))
                    for t4 in range(4):
                        o_ps = psum.tile([128, 512], f32, tag="acc", bufs=4,
                                         name="o_ps")
                        for k in range(KC):
                            nc.tensor.matmul(
                                o_ps[:],
                                lhsT=att_sb[:, k, 128 * t4:128 * (t4 + 1)],
                                rhs=wo_sb[:, k, :],
                                start=(k == 0), stop=(k == KC - 1))
                        o_sb = ph4.tile([128, 512], f32, tag="ostage", bufs=3,
                                        name="o_sb")
                        nc.scalar.activation(o_sb[:], o_ps[:], Copy)
                        nc.sync.dma_start(
                            out=out[128 * t4:128 * (t4 + 1),
                                    512 * n8:512 * (n8 + 1)],
                            in_=o_sb[:])
    nc.finalize()
    return nc


def kernel(hidden_states, Wq, Wkv, Wo):
    global LAST_EXEC_NS
    bf = ml_dtypes.bfloat16
    hs = np.asarray(hidden_states, dtype=np.float32)
    Wq = np.asarray(Wq, dtype=np.float32)
    Wkv_np = np.asarray(Wkv, dtype=np.float32)
    Wo = np.asarray(Wo, dtype=np.float32)

    hs_flat = hs.reshape(B * S, H)
    Wq_t = np.ascontiguousarray(
        Wq.reshape(H, 32, 128).transpose(1, 0, 2)).astype(bf)
    Wo_t = np.ascontiguousarray(
        Wo.reshape(H, 8, 512).transpose(1, 0, 2)).astype(bf)
    Wkv_bf = Wkv_np.astype(bf)
    rel = _build_rel()
    slopes = _alibi_slopes(NH)

    in_maps = []
    for c in range(NC):
        hsT_c = np.ascontiguousarray(
            hs_flat[TPC * c:TPC * (c + 1)].T).astype(bf)
        slopes_c = np.ascontiguousarray(
            np.broadcast_to(slopes[4 * c:4 * c + 4][None, :], (128, 4)))
        in_maps.append({
            "hsT": hsT_c, "Wq_t": Wq_t, "Wkv": Wkv_bf, "Wo_t": Wo_t,
            "rel": rel, "slopes": slopes_c,
        })

    if "nc" not in _CACHE:
        _CACHE["nc"] = _build_nc()
    nc = _CACHE["nc"]
    trace = bool(int(os.environ.get("BASS_KERNEL_TRACE", "0")))
    res = run_bass_kernel_spmd(nc, in_maps, core_ids=list(range(NC)),
                               trace=trace)
    LAST_EXEC_NS = res.exec_time_ns
    out = np.concatenate([res.results[c]["out"] for c in range(NC)], axis=0)
    return out.reshape(B, S, H).astype(np.float32)


# revision 5
# speedup vs baseline: 1.0705x; 1.0705x over previous
"""MQA attention with ALiBi + causal mask on 8 TRN2 NeuronCores.

Problem: hidden_states [2,2048,4096] @ Wq -> 32 query heads of 128; single
KV head via Wkv; scores + ALiBi bias + causal mask; softmax; @ Wo.

Distribution (avoids the 64 MiB AllReduce of plain head-TP):
- Tokens flattened batch-major: t = b*2048 + s. Core c owns tokens
  [512c, 512(c+1)) for all projections; output rows are disjoint and the
  host just concatenates.
- Attention is head-sharded round-robin (core c gets heads {c, c+8, c+16,
  c+24}) so every core's causal + ALiBi-cutoff workload is identical.
  Shardings are bridged by two 4 MiB-per-core bf16 AllToAlls (qT in,
  attnT out), each split into token-halves so the second half transfers
  while attention / output-projection computes, plus AllGathers of the
  tiny single-head K/V.
- ALiBi distance cutoff: a kv chunk whose distance exceeds 45/slope
  contributes < 1e-13 of the softmax mass and is skipped. Per-slot
  (head-octile) uniform bounds keep the SPMD program identical per core.
- All matmuls run in bf16 (rel-err budget 2e-2); softmax in f32 without
  max-subtraction (scores are O(10), bias <= 0 -> exp never overflows and
  the causal diagonal keeps denominators O(1)).

Softmax layout: scores built transposed (scoresT[kpos, q]) so probs @ V
needs no transposes; denominator via ones-stationary matmul accumulated
alongside; per-q reciprocal broadcast across partitions by a K=1
outer-product matmul.

Weights are pre-tiled on the host into SBUF layout (partition-major
[128, k, n]) so every weight DMA is fully contiguous.
"""
import math
import os

import numpy as np
import ml_dtypes

import concourse.bass as bass
from concourse import bacc
import concourse.mybir as mybir
from concourse.tile import TileContext
from concourse.bass_utils import run_bass_kernel_spmd

B, S, H, NH, HD = 2, 2048, 4096, 32, 128
NC = 8              # cores
TPC = 512           # tokens per core
KC = H // 128       # 32 contraction chunks
GQ = 8              # 256-token q blocks per batch
NREL = GQ * (GQ + 1)  # 72 (g, j) rel tiles
SCALE = HD ** -0.5
# per-slot ALiBi reach (slot s = head octile): ceil(45 / slope) maxed over octile
SLOT_D = [180.0, 1440.0, float("inf"), float("inf")]
bf16 = mybir.dt.bfloat16
f32 = mybir.dt.float32
Exp = mybir.ActivationFunctionType.Exp
Copy = mybir.ActivationFunctionType.Copy
MULT = mybir.AluOpType.mult
ADD = mybir.AluOpType.add

_CACHE = {}
LAST_EXEC_NS = None


def _alibi_slopes(n_heads):
    closest_pow2 = 2 ** math.floor(math.log2(n_heads))
    base = 2.0 ** (-(2.0 ** -(math.log2(closest_pow2) - 3)))
    slopes = [base ** i for i in range(1, closest_pow2 + 1)]
    if closest_pow2 != n_heads:
        extra_base = 2.0 ** (-(2.0 ** -(math.log2(2 * closest_pow2) - 3)))
        n_extra = min(closest_pow2, n_heads - closest_pow2)
        slopes += [extra_base ** i for i in range(1, 2 * n_extra + 1, 2)]
    return np.asarray(slopes, dtype=np.float32)


def _j0(g, slot):
    d = SLOT_D[slot]
    if math.isinf(d):
        return 0
    return max(0, math.ceil((256 * g - 127 - d) / 128))


def _rel_idx(g, j):
    return g * g + g + j


def _build_rel():
    rel = np.empty((128, NREL * 256), np.float32)
    for g in range(GQ):
        for j in range(2 * g + 2):
            kpos = 128 * j + np.arange(128)[:, None]
            q = 256 * g + np.arange(256)[None, :]
            r = (kpos - q).astype(np.float32)
            r[kpos > q] = -30000.0
            i = _rel_idx(g, j)
            rel[:, 256 * i:256 * (i + 1)] = r
    return rel.astype(ml_dtypes.bfloat16)


def _build_nc():
    nc = bacc.Bacc(num_devices=NC)
    # host-pre-tiled layouts: every DMA below is contiguous in DRAM
    hsT = nc.declare_dram_parameter("hsT", [128, KC * TPC], bf16, isOutput=False)
    Wq_t = nc.declare_dram_parameter("Wq_t", [32, 128, KC * 128], bf16, isOutput=False)
    Wkv = nc.declare_dram_parameter("Wkv", [128, KC * 256], bf16, isOutput=False)
    Wo_t = nc.declare_dram_parameter("Wo_t", [8, 128, KC * 512], bf16, isOutput=False)
    rel = nc.declare_dram_parameter("rel", [128, NREL * 256], bf16, isOutput=False)
    slopes = nc.declare_dram_parameter("slopes", [128, 4], f32, isOutput=False)
    out = nc.declare_dram_parameter("out", [TPC, H], f32, isOutput=True)

    grp = [list(range(NC))]
    with TileContext(nc) as tc:
        with (
            tc.tile_pool(name="dram", bufs=1, space="DRAM") as dram,
            tc.tile_pool(name="const", bufs=1) as const,
            tc.tile_pool(name="psum", bufs=1, space="PSUM") as psum,
        ):
            kT_in = dram.tile([128, TPC], bf16)
            kT_ag = dram.tile([128 * NC, TPC], bf16, addr_space="Shared")
            v_in = dram.tile([TPC, 128], bf16)
            v_ag = dram.tile([TPC * NC, 128], bf16, addr_space="Shared")
            q_in = [dram.tile([H, 256], bf16, name=f"q_in{p}") for p in range(2)]
            q_a2a = [dram.tile([H, 256], bf16, name=f"q_a2a{p}") for p in range(2)]
            a_in = [dram.tile([H, 256], bf16, name=f"a_in{p}") for p in range(2)]
            a_a2a = [dram.tile([H, 256], bf16, name=f"a_a2a{p}") for p in range(2)]

            rel_sb = const.tile([128, NREL * 256], bf16)
            nc.sync.dma_start(out=rel_sb[:], in_=rel[:])
            slopes_sb = const.tile([128, 4], f32)
            nc.sync.dma_start(out=slopes_sb[:], in_=slopes[:])
            ones_col = const.tile([128, 1], bf16)
            nc.vector.memset(ones_col[:], 1.0)
            ones_row = const.tile([1, 128], bf16)
            nc.vector.memset(ones_row[:], 1.0)

            # ---------------- Phase 1: q/k/v projections -------------------
            with tc.tile_pool(name="ph1", bufs=1) as ph1:
                hsT_sb = ph1.tile([128, KC, TPC], bf16)
                nc.sync.dma_start(out=hsT_sb[:],
                                  in_=hsT.rearrange("p (k t) -> p k t", k=KC))
                Wkv_sb = ph1.tile([128, KC, 256], bf16)
                nc.sync.dma_start(out=Wkv_sb[:],
                                  in_=Wkv.rearrange("p (k c) -> p k c", k=KC))

                kT_ps = psum.tile([128, TPC], f32, tag="str", bufs=3)
                for k in range(KC):
                    nc.tensor.matmul(kT_ps[:], lhsT=Wkv_sb[:, k, 0:128],
                                     rhs=hsT_sb[:, k, :],
                                     start=(k == 0), stop=(k == KC - 1))
                kT_sb = ph1.tile([128, TPC], bf16)
                nc.vector.tensor_copy(out=kT_sb[:], in_=kT_ps[:])
                nc.sync.dma_start(out=kT_in[:], in_=kT_sb[:])

                for t4 in range(4):
                    v_ps = psum.tile([128, 128], f32, tag="str", bufs=3,
                                     name="v_ps")
                    for k in range(KC):
                        nc.tensor.matmul(
                            v_ps[:], lhsT=hsT_sb[:, k, 128 * t4:128 * (t4 + 1)],
                            rhs=Wkv_sb[:, k, 128:256],
                            start=(k == 0), stop=(k == KC - 1))
                    v_sb = ph1.tile([128, 128], bf16, tag="v_sb", bufs=3,
                                    name="v_sb")
                    nc.vector.tensor_copy(out=v_sb[:], in_=v_ps[:])
                    nc.sync.dma_start(out=v_in[128 * t4:128 * (t4 + 1), :],
                                      in_=v_sb[:])

                # K/V AllGathers issue as soon as the tiny kv DMAs land,
                # overlapping the whole q projection below.
                nc.gpsimd.collective_compute(
                    "AllGather", mybir.AluOpType.bypass, replica_groups=grp,
                    ins=[kT_in[:]], outs=[kT_ag[:]])
                nc.gpsimd.collective_compute(
                    "AllGather", mybir.AluOpType.bypass, replica_groups=grp,
                    ins=[v_in[:]], outs=[v_ag[:]])

                for dq in range(32):
                    wq_sb = ph1.tile([128, KC, 128], bf16, tag="wq", bufs=3,
                                     name="wq_sb")
                    nc.sync.dma_start(
                        out=wq_sb[:],
                        in_=Wq_t[dq].rearrange("p (k m) -> p k m", k=KC))
                    q_ps = psum.tile([128, TPC], f32, tag="acc", bufs=3,
                                     name="q_ps")
                    for k in range(KC):
                        nc.tensor.matmul(q_ps[:], lhsT=wq_sb[:, k, :],
                                         rhs=hsT_sb[:, k, :],
                                         start=(k == 0), stop=(k == KC - 1))
                    q_sb = ph1.tile([128, TPC], bf16, tag="qstage", bufs=3,
                                    name="q_sb")
                    nc.vector.tensor_scalar_mul(q_sb[:], q_ps[:], SCALE)
                    # head dq -> rank dq%8, slot dq//8 (round-robin heads)
                    row = 512 * (dq % 8) + 128 * (dq // 8)
                    for p in range(2):
                        nc.sync.dma_start(
                            out=q_in[p][row:row + 128, :],
                            in_=q_sb[:, 256 * p:256 * (p + 1)])

            for p in range(2):
                nc.gpsimd.collective_compute(
                    "AllToAll", mybir.AluOpType.bypass, replica_groups=grp,
                    ins=[q_in[p][:]], outs=[q_a2a[p][:]])

            # ---------------- Phase 3: attention for my 4 heads ------------
            # my slot-s head: global head = c + 8s; slope from slopes_sb[:, s]
            with tc.tile_pool(name="attn", bufs=1) as attn:
                qT = {}
                aT = {}
                for b in range(B):
                    for s in range(4):
                        for p in range(2):
                            t = attn.tile([128, 4, 256], bf16,
                                          name=f"qT_{b}_{s}_{p}")
                            nc.sync.dma_start(
                                out=t[:],
                                in_=q_a2a[p].rearrange(
                                    "(j s p) t -> s p j t", s=4,
                                    p=128)[s][:, 4 * b:4 * b + 4, :])
                            qT[b, s, p] = t
                            aT[b, s, p] = attn.tile([128, 4, 256], bf16,
                                                    name=f"aT_{b}_{s}_{p}")
                kT_b = {}
                v_b = {}
                for b in range(B):
                    t = attn.tile([128, 4, TPC], bf16, name=f"kT_{b}")
                    nc.sync.dma_start(
                        out=t[:],
                        in_=kT_ag.rearrange("(s p) t -> p s t",
                                            p=128)[:, 4 * b:4 * b + 4, :])
                    kT_b[b] = t
                    t = attn.tile([128, 16, 128], bf16, name=f"v_{b}")
                    nc.sync.dma_start(
                        out=t[:],
                        in_=v_ag.rearrange("(b j p) d -> b p j d",
                                           b=2, p=128)[b])
                    v_b[b] = t

                for par in range(2):
                    for b in range(B):
                        for hp in range(2):
                            slots = (2 * hp, 2 * hp + 1)
                            for g in range(par, GQ, 2):
                                gs, go = g // 2, 256 * (g % 2)
                                j0 = [_j0(g, s) for s in slots]
                                jlo = min(j0)
                                nch = 2 * (g + 1)
                                at = [psum.tile([128, 256], f32, tag="acc",
                                                bufs=3, name=f"at{hi}")
                                      for hi in range(2)]
                                den = [psum.tile([1, 256], f32, tag="den",
                                                 bufs=2, name=f"den{hi}")
                                       for hi in range(2)]
                                for j in range(jlo, nch):
                                    active = [hi for hi in range(2)
                                              if j >= j0[hi]]
                                    expj = attn.tile([128, 512], bf16,
                                                     tag="exp", bufs=4,
                                                     name="expj")
                                    for hi in active:
                                        s_ps = psum.tile([128, 256], f32,
                                                         tag="str", bufs=3,
                                                         name="s_ps")
                                        nc.tensor.matmul(
                                            s_ps[:],
                                            lhsT=kT_b[b][:, j // 4,
                                                         128 * (j % 4):128 * (j % 4 + 1)],
                                            rhs=qT[b, slots[hi], g % 2][:, gs, :],
                                            start=True, stop=True)
                                        tmp = attn.tile([128, 256], f32,
                                                        tag="stt", bufs=3,
                                                        name="tmp")
                                        i0 = 256 * _rel_idx(g, j)
                                        nc.vector.scalar_tensor_tensor(
                                            out=tmp[:],
                                            in0=rel_sb[:, i0:i0 + 256],
                                            scalar=slopes_sb[:, slots[hi]:slots[hi] + 1],
                                            in1=s_ps[:], op0=MULT, op1=ADD)
                                        nc.scalar.activation(
                                            expj[:, 256 * hi:256 * (hi + 1)],
                                            tmp[:], Exp)
                                        nc.tensor.matmul(
                                            at[hi][:], lhsT=v_b[b][:, j, :],
                                            rhs=expj[:, 256 * hi:256 * (hi + 1)],
                                            start=(j == j0[hi]),
                                            stop=(j == nch - 1))
                                        nc.tensor.matmul(
                                            den[hi][:],
                                            lhsT=ones_col[:],
                                            rhs=expj[:, 256 * hi:256 * (hi + 1)],
                                            start=(j == j0[hi]),
                                            stop=(j == nch - 1))
                                den_sb = attn.tile([1, 512], f32, tag="den_sb",
                                                   bufs=2, name="den_sb")
                                for hi in range(2):
                                    nc.vector.tensor_copy(
                                        out=den_sb[0:1, 256 * hi:256 * (hi + 1)],
                                        in_=den[hi][:])
                                rec = attn.tile([1, 512], f32, tag="rec",
                                                bufs=2, name="rec")
                                nc.vector.reciprocal_approx_fast(out=rec[:],
                                                                 in_=den_sb[:])
                                rec_bf = attn.tile([1, 512], bf16, tag="rec_bf",
                                                   bufs=2, name="rec_bf")
                                nc.vector.tensor_copy(out=rec_bf[:], in_=rec[:])
                                rb_ps = psum.tile([128, 512], f32, tag="str",
                                                  bufs=3, name="rb_ps")
                                nc.tensor.matmul(rb_ps[:], lhsT=ones_row[:],
                                                 rhs=rec_bf[:], start=True,
                                                 stop=True)
                                rb_sb = attn.tile([128, 512], f32, tag="rb",
                                                  bufs=2, name="rb_sb")
                                nc.scalar.activation(rb_sb[:], rb_ps[:], Copy)
                                for hi in range(2):
                                    nc.vector.tensor_tensor(
                                        out=aT[b, slots[hi], g % 2][:, gs, :],
                                        in0=at[hi][:],
                                        in1=rb_sb[:, 256 * hi:256 * (hi + 1)],
                                        op=MULT)
                    # ship this parity's attnT as soon as it's done
                    for b in range(B):
                        for s in range(4):
                            nc.sync.dma_start(
                                out=a_in[par].rearrange(
                                    "(j s p) t -> s p j t", s=4,
                                    p=128)[s][:, 4 * b:4 * b + 4, :],
                                in_=aT[b, s, par][:])
                    nc.gpsimd.collective_compute(
                        "AllToAll", mybir.AluOpType.bypass, replica_groups=grp,
                        ins=[a_in[par][:]], outs=[a_a2a[par][:]])

            # ---------------- Phase 4: output projection -------------------
            # a_a2a[par] rows [512j + 128i] hold global head (j + 8i) for my
            # 256 par-tokens; contraction chunk k (head k) sits at l(k).
            with tc.tile_pool(name="ph4", bufs=1) as ph4:
                att_sb = {}
                for par in range(2):
                    att_sb[par] = ph4.tile([128, KC, 256], bf16,
                                           name=f"att_sb{par}")
                    nc.sync.dma_start(
                        out=att_sb[par][:],
                        in_=a_a2a[par].rearrange("(l p) t -> p l t", p=128))
                for n8 in range(8):
                    wo_sb = ph4.tile([128, KC, 512], bf16, tag="wo", bufs=2,
                                     name="wo_sb")
                    nc.sync.dma_start(
                        out=wo_sb[:],
                        in_=Wo_t[n8].rearrange("p (k n) -> p k n", k=KC))
                    for t4 in range(4):
                        par, half = t4 // 2, t4 % 2
                        o_ps = psum.tile([128, 512], f32, tag="acc", bufs=3,
                                         name="o_ps")
                        for k in range(KC):
                            l = 4 * (k % 8) + (k // 8)
                            nc.tensor.matmul(
                                o_ps[:],
                                lhsT=att_sb[par][:, l, 128 * half:128 * (half + 1)],
                                rhs=wo_sb[:, k, :],
                                start=(k == 0), stop=(k == KC - 1))
                        o_sb = ph4.tile([128, 512], f32, tag="ostage", bufs=3,
                                        name="o_sb")
                        nc.vector.tensor_copy(out=o_sb[:], in_=o_ps[:])
                        # token row: par half of my 512 tokens
                        r0 = 256 * par + 128 * half
                        nc.sync.dma_start(
                            out=out[r0:r0 + 128, 512 * n8:512 * (n8 + 1)],
                            in_=o_sb[:])
    nc.finalize()
    return nc


def kernel(hidden_states, Wq, Wkv, Wo):
    global LAST_EXEC_NS
    bf = ml_dtypes.bfloat16
    hs = np.asarray(hidden_states, dtype=np.float32)
    Wq = np.asarray(Wq, dtype=np.float32)
    Wkv_np = np.asarray(Wkv, dtype=np.float32)
    Wo = np.asarray(Wo, dtype=np.float32)

    hs_flat = hs.reshape(B * S, H)
    # pre-tile into SBUF partition-major layouts (all DMAs contiguous)
    Wq_t = np.ascontiguousarray(
        Wq.reshape(KC, 128, 32, 128).transpose(2, 1, 0, 3)
        .reshape(32, 128, KC * 128)).astype(bf)
    Wo_t = np.ascontiguousarray(
        Wo.reshape(KC, 128, 8, 512).transpose(2, 1, 0, 3)
        .reshape(8, 128, KC * 512)).astype(bf)
    Wkv_t = np.ascontiguousarray(
        Wkv_np.reshape(KC, 128, 256).transpose(1, 0, 2)
        .reshape(128, KC * 256)).astype(bf)
    rel = _build_rel()
    slopes = _alibi_slopes(NH)

    in_maps = []
    for c in range(NC):
        blk = hs_flat[TPC * c:TPC * (c + 1)]          # [512, 4096]
        hsT_c = np.ascontiguousarray(
            blk.T.reshape(KC, 128, TPC).transpose(1, 0, 2)
            .reshape(128, KC * TPC)).astype(bf)
        my_heads = [c + 8 * s for s in range(4)]
        slopes_c = np.ascontiguousarray(
            np.broadcast_to(slopes[my_heads][None, :], (128, 4)))
        in_maps.append({
            "hsT": hsT_c, "Wq_t": Wq_t, "Wkv": Wkv_t, "Wo_t": Wo_t,
            "rel": rel, "slopes": slopes_c,
        })

    if "nc" not in _CACHE:
        _CACHE["nc"] = _build_nc()
    nc = _CACHE["nc"]
    trace = bool(int(os.environ.get("BASS_KERNEL_TRACE", "0")))
    res = run_bass_kernel_spmd(nc, in_maps, core_ids=list(range(NC)),
                               trace=trace)
    LAST_EXEC_NS = res.exec_time_ns
    out = np.concatenate([res.results[c]["out"] for c in range(NC)], axis=0)
    return out.reshape(B, S, H).astype(np.float32)


# revision 8
# speedup vs baseline: 1.1160x; 1.0425x over previous
"""MQA attention with ALiBi + causal mask on 8 TRN2 NeuronCores.

Problem: hidden_states [2,2048,4096] @ Wq -> 32 query heads of 128; single
KV head via Wkv; scores + ALiBi bias + causal mask; softmax; @ Wo.

Distribution (avoids the 64 MiB AllReduce of plain head-TP):
- Core c owns tokens [256c, 256(c+1)) of BOTH batches for all projections
  (output rows disjoint -> host concatenates). Attention is head-sharded
  round-robin (core c gets heads {c, c+8, c+16, c+24}) so every core's
  causal + ALiBi-cutoff workload is identical. The two shardings are
  bridged by per-batch bf16 AllToAlls of qT / attnT (2 MiB per core each)
  plus AllGathers of the tiny single-head K/V. Splitting everything by
  batch pipelines the phases: batch-1 attention (DVE/ACT-bound) runs
  while batch-0 output projection (PE-bound) executes.
- ALiBi distance cutoff: a kv chunk whose distance exceeds 45/slope
  contributes < 1e-13 of the softmax mass and is skipped. Per-slot
  (head-octile) uniform bounds keep the SPMD program identical per core.
- All matmuls in bf16 (rel-err budget 2e-2); softmax in f32 without
  max-subtraction (scores are O(10), bias <= 0 -> exp never overflows and
  the causal diagonal keeps denominators O(1)).

Softmax layout: scores are built transposed (scoresT[kpos, q]) so the
probs @ V matmul needs no transposes; kv chunks are processed in pairs
(one [128,512] bias-FMA + exp per pair); the denominator comes from
ones-stationary matmuls accumulated alongside and the per-q reciprocal is
broadcast across partitions with a K=1 outer-product matmul. The ALiBi
rel/mask tile depends only on delta = j - 2g, so 16 distinct [128,256]
tiles cover all (g, j).

Weights are pre-tiled on the host into SBUF partition-major layouts so
every weight DMA is fully contiguous.
"""
import math
import os

import numpy as np
import ml_dtypes

import concourse.bass as bass
from concourse import bacc
import concourse.mybir as mybir
from concourse.tile import TileContext
from concourse.bass_utils import run_bass_kernel_spmd

B, S, H, NH, HD = 2, 2048, 4096, 32, 128
NC = 8              # cores
TPC = 512           # tokens per core (256 per batch)
KC = H // 128       # 32 contraction chunks
GQ = 8              # 256-token q blocks per batch
SCALE = HD ** -0.5
# per-slot ALiBi reach (slot s = head octile): 45/slope maxed over octile
SLOT_D = [180.0, 1440.0, float("inf"), float("inf")]
bf16 = mybir.dt.bfloat16
f32 = mybir.dt.float32
Exp = mybir.ActivationFunctionType.Exp
Copy = mybir.ActivationFunctionType.Copy
MULT = mybir.AluOpType.mult
ADD = mybir.AluOpType.add

_CACHE = {}
LAST_EXEC_NS = None


def _alibi_slopes(n_heads):
    closest_pow2 = 2 ** math.floor(math.log2(n_heads))
    base = 2.0 ** (-(2.0 ** -(math.log2(closest_pow2) - 3)))
    slopes = [base ** i for i in range(1, closest_pow2 + 1)]
    if closest_pow2 != n_heads:
        extra_base = 2.0 ** (-(2.0 ** -(math.log2(2 * closest_pow2) - 3)))
        n_extra = min(closest_pow2, n_heads - closest_pow2)
        slopes += [extra_base ** i for i in range(1, 2 * n_extra + 1, 2)]
    return np.asarray(slopes, dtype=np.float32)


def _j0(g, slot):
    d = SLOT_D[slot]
    if math.isinf(d):
        return 0
    return max(0, math.ceil((256 * g - 127 - d) / 128))


def _build_rel():
    # tile for delta = j - 2g at slice index (delta + 14): rel = 128*delta + p - f
    rel = np.empty((128, 16 * 256), np.float32)
    p = np.arange(128)[:, None]
    f = np.arange(256)[None, :]
    for idx in range(16):
        delta = idx - 14
        r = (128 * delta + p - f).astype(np.float32)
        r[128 * delta + p - f > 0] = -30000.0
        rel[:, 256 * idx:256 * (idx + 1)] = r
    return rel.astype(ml_dtypes.bfloat16)


def _build_nc():
    nc = bacc.Bacc(num_devices=NC)
    # host-pre-tiled layouts: every DMA below is contiguous in DRAM
    hsT = nc.declare_dram_parameter("hsT", [128, KC * TPC], bf16, isOutput=False)
    Wq_t = nc.declare_dram_parameter("Wq_t", [32, 128, KC * 128], bf16, isOutput=False)
    Wkv = nc.declare_dram_parameter("Wkv", [128, KC * 256], bf16, isOutput=False)
    Wo_t = nc.declare_dram_parameter("Wo_t", [8, 128, KC * 512], bf16, isOutput=False)
    rel = nc.declare_dram_parameter("rel", [128, 16 * 256], bf16, isOutput=False)
    slopes = nc.declare_dram_parameter("slopes", [128, 4], f32, isOutput=False)
    out = nc.declare_dram_parameter("out", [TPC, H], f32, isOutput=True)

    grp = [list(range(NC))]
    with TileContext(nc) as tc:
        with (
            tc.tile_pool(name="dram", bufs=1, space="DRAM") as dram,
            tc.tile_pool(name="const", bufs=1) as const,
            tc.tile_pool(name="psum", bufs=1, space="PSUM") as psum,
        ):
            kT_in = dram.tile([128, TPC], bf16)
            kT_ag = dram.tile([128 * NC, TPC], bf16, addr_space="Shared")
            v_in = dram.tile([TPC, 128], bf16)
            v_ag = dram.tile([TPC * NC, 128], bf16, addr_space="Shared")
            q_in = [dram.tile([H, 256], bf16, name=f"q_in{b}") for b in range(2)]
            q_a2a = [dram.tile([H, 256], bf16, name=f"q_a2a{b}") for b in range(2)]
            a_in = [dram.tile([H, 256], bf16, name=f"a_in{b}") for b in range(2)]
            a_a2a = [dram.tile([H, 256], bf16, name=f"a_a2a{b}") for b in range(2)]

            rel_sb = const.tile([128, 16 * 256], bf16)
            nc.sync.dma_start(out=rel_sb[:], in_=rel[:])
            slopes_sb = const.tile([128, 4], f32)
            nc.sync.dma_start(out=slopes_sb[:], in_=slopes[:])
            ones_col = const.tile([128, 1], bf16)
            nc.vector.memset(ones_col[:], 1.0)
            ones_row = const.tile([1, 128], bf16)
            nc.vector.memset(ones_row[:], 1.0)

            # ---------------- Phase 1: q/k/v projections -------------------
            with tc.tile_pool(name="ph1", bufs=1) as ph1:
                hsT_sb = ph1.tile([128, KC, TPC], bf16)
                nc.sync.dma_start(out=hsT_sb[:],
                                  in_=hsT.rearrange("p (k t) -> p k t", k=KC))
                Wkv_sb = ph1.tile([128, KC, 256], bf16)
                nc.sync.dma_start(out=Wkv_sb[:],
                                  in_=Wkv.rearrange("p (k c) -> p k c", k=KC))

                kT_ps = psum.tile([128, TPC], f32, tag="str", bufs=3)
                for k in range(KC):
                    nc.tensor.matmul(kT_ps[:], lhsT=Wkv_sb[:, k, 0:128],
                                     rhs=hsT_sb[:, k, :],
                                     start=(k == 0), stop=(k == KC - 1))
                kT_sb = ph1.tile([128, TPC], bf16)
                nc.vector.tensor_copy(out=kT_sb[:], in_=kT_ps[:])
                nc.sync.dma_start(out=kT_in[:], in_=kT_sb[:])

                for t4 in range(4):
                    v_ps = psum.tile([128, 128], f32, tag="str", bufs=3,
                                     name="v_ps")
                    for k in range(KC):
                        nc.tensor.matmul(
                            v_ps[:], lhsT=hsT_sb[:, k, 128 * t4:128 * (t4 + 1)],
                            rhs=Wkv_sb[:, k, 128:256],
                            start=(k == 0), stop=(k == KC - 1))
                    v_sb = ph1.tile([128, 128], bf16, tag="v_sb", bufs=3,
                                    name="v_sb")
                    nc.vector.tensor_copy(out=v_sb[:], in_=v_ps[:])
                    nc.sync.dma_start(out=v_in[128 * t4:128 * (t4 + 1), :],
                                      in_=v_sb[:])

                # K/V AllGathers issue as soon as the tiny kv DMAs land,
                # overlapping the whole q projection below.
                nc.gpsimd.collective_compute(
                    "AllGather", mybir.AluOpType.bypass, replica_groups=grp,
                    ins=[kT_in[:]], outs=[kT_ag[:]])
                nc.gpsimd.collective_compute(
                    "AllGather", mybir.AluOpType.bypass, replica_groups=grp,
                    ins=[v_in[:]], outs=[v_ag[:]])

                for dq in range(32):
                    wq_sb = ph1.tile([128, KC, 128], bf16, tag="wq", bufs=3,
                                     name="wq_sb")
                    nc.sync.dma_start(
                        out=wq_sb[:],
                        in_=Wq_t[dq].rearrange("p (k m) -> p k m", k=KC))
                    q_ps = psum.tile([128, TPC], f32, tag="acc", bufs=3,
                                     name="q_ps")
                    for k in range(KC):
                        nc.tensor.matmul(q_ps[:], lhsT=wq_sb[:, k, :],
                                         rhs=hsT_sb[:, k, :],
                                         start=(k == 0), stop=(k == KC - 1))
                    q_sb = ph1.tile([128, TPC], bf16, tag="qstage", bufs=3,
                                    name="q_sb")
                    nc.vector.tensor_scalar_mul(q_sb[:], q_ps[:], SCALE)
                    # head dq -> rank dq%8, slot dq//8 (round-robin heads)
                    row = 512 * (dq % 8) + 128 * (dq // 8)
                    for b in range(2):
                        nc.sync.dma_start(
                            out=q_in[b][row:row + 128, :],
                            in_=q_sb[:, 256 * b:256 * (b + 1)])

            for b in range(2):
                nc.gpsimd.collective_compute(
                    "AllToAll", mybir.AluOpType.bypass, replica_groups=grp,
                    ins=[q_in[b][:]], outs=[q_a2a[b][:]])

            # ---------------- Phases 3+4: attention & output projection ----
            # my slot-s head: global head = c + 8s; slope = slopes_sb[:, s]
            with (tc.tile_pool(name="attn", bufs=1) as attn,
                  tc.tile_pool(name="ph4", bufs=1) as ph4):
                kT_b, v_b, qT, aT = {}, {}, {}, {}
                for b in range(B):
                    t = attn.tile([128, 8, 256], bf16, name=f"kT_{b}")
                    nc.sync.dma_start(
                        out=t[:],
                        in_=kT_ag.rearrange("(r p) (b t) -> b p r t",
                                            p=128, b=2)[b])
                    kT_b[b] = t
                    t = attn.tile([128, 8, 2, 128], bf16, name=f"v_{b}")
                    for u in range(2):
                        nc.sync.dma_start(
                            out=t[:, :, u, :],
                            in_=v_ag.rearrange("(r b u p) d -> b p r u d",
                                               b=2, u=2, p=128)[b][:, :, u, :])
                    v_b[b] = t

                def kT_chunk(b, j):
                    return kT_b[b][:, j // 2, 128 * (j % 2):128 * (j % 2 + 1)]

                for b in range(B):
                    for s in range(4):
                        t = attn.tile([128, 8, 256], bf16, tag="qT", bufs=8,
                                      name=f"qT_{b}_{s}")
                        nc.sync.dma_start(
                            out=t[:],
                            in_=q_a2a[b].rearrange("(j s p) t -> s p j t",
                                                   s=4, p=128)[s])
                        qT[b, s] = t
                        aT[b, s] = attn.tile([128, 8, 256], bf16, tag="aT",
                                             bufs=6, name=f"aT_{b}_{s}")

                def do_attention(b):
                    for hp in range(2):
                        slots = (2 * hp, 2 * hp + 1)
                        for g in range(GQ):
                            nch = 2 * (g + 1)
                            j0 = [_j0(g, s) for s in slots]
                            at = [psum.tile([128, 256], f32, tag="acc",
                                            bufs=3, name=f"at{hi}")
                                  for hi in range(2)]
                            den = [psum.tile([1, 256], f32, tag="den",
                                             bufs=2, name=f"den{hi}")
                                   for hi in range(2)]
                            for hi in range(2):
                                sl = slots[hi]
                                js = list(range(j0[hi], nch))
                                pairs = [(js[i], js[i + 1] if i + 1 < len(js)
                                          else None)
                                         for i in range(0, len(js), 2)]
                                for ja, jb in pairs:
                                    w = 512 if jb is not None else 256
                                    s2 = psum.tile([128, 512], f32, tag="str",
                                                   bufs=3, name="s2")
                                    nc.tensor.matmul(
                                        s2[:, 0:256], lhsT=kT_chunk(b, ja),
                                        rhs=qT[b, sl][:, g, :],
                                        start=True, stop=True)
                                    if jb is not None:
                                        nc.tensor.matmul(
                                            s2[:, 256:512],
                                            lhsT=kT_chunk(b, jb),
                                            rhs=qT[b, sl][:, g, :],
                                            start=True, stop=True)
                                    tmp = attn.tile([128, 512], f32, tag="stt",
                                                    bufs=3, name="tmp")
                                    d0 = 256 * (ja - 2 * g + 14)
                                    nc.vector.scalar_tensor_tensor(
                                        out=tmp[:, 0:w],
                                        in0=rel_sb[:, d0:d0 + w],
                                        scalar=slopes_sb[:, sl:sl + 1],
                                        in1=s2[:, 0:w], op0=MULT, op1=ADD)
                                    expp = attn.tile([128, 512], bf16,
                                                     tag="exp", bufs=4,
                                                     name="expp")
                                    nc.scalar.activation(expp[:, 0:w],
                                                         tmp[:, 0:w], Exp)
                                    for ji, j in enumerate((ja, jb)):
                                        if j is None:
                                            continue
                                        e_sl = expp[:, 256 * ji:256 * (ji + 1)]
                                        nc.tensor.matmul(
                                            at[hi][:], lhsT=v_b[b][:, j // 2, j % 2, :],
                                            rhs=e_sl,
                                            start=(j == j0[hi]),
                                            stop=(j == nch - 1))
                                        nc.tensor.matmul(
                                            den[hi][:], lhsT=ones_col[:],
                                            rhs=e_sl,
                                            start=(j == j0[hi]),
                                            stop=(j == nch - 1))
                            rec = attn.tile([1, 512], f32, tag="rec", bufs=2,
                                            name="rec")
                            for hi in range(2):
                                nc.vector.reciprocal_approx_fast(
                                    out=rec[0:1, 256 * hi:256 * (hi + 1)],
                                    in_=den[hi][:])
                            rec_bf = attn.tile([1, 512], bf16, tag="rec_bf",
                                               bufs=2, name="rec_bf")
                            nc.vector.tensor_copy(out=rec_bf[:], in_=rec[:])
                            rb_ps = psum.tile([128, 512], f32, tag="str",
                                              bufs=3, name="rb_ps")
                            nc.tensor.matmul(rb_ps[:], lhsT=ones_row[:],
                                             rhs=rec_bf[:], start=True,
                                             stop=True)
                            rb_sb = attn.tile([128, 512], f32, tag="rb",
                                              bufs=2, name="rb_sb")
                            nc.scalar.activation(rb_sb[:], rb_ps[:], Copy)
                            for hi in range(2):
                                nc.vector.tensor_tensor(
                                    out=aT[b, slots[hi]][:, g, :],
                                    in0=at[hi][:],
                                    in1=rb_sb[:, 256 * hi:256 * (hi + 1)],
                                    op=MULT)
                    for s in range(4):
                        nc.sync.dma_start(
                            out=a_in[b].rearrange("(j s p) t -> s p j t",
                                                  s=4, p=128)[s],
                            in_=aT[b, s][:])
                    nc.gpsimd.collective_compute(
                        "AllToAll", mybir.AluOpType.bypass, replica_groups=grp,
                        ins=[a_in[b][:]], outs=[a_a2a[b][:]])

                do_attention(0)
                do_attention(1)

                # output projection; batch-0 halves overlap batch-1 attention
                att_sb = {}
                for b in range(2):
                    att_sb[b] = ph4.tile([128, KC, 256], bf16,
                                         name=f"att_sb{b}")
                    nc.sync.dma_start(
                        out=att_sb[b][:],
                        in_=a_a2a[b].rearrange("(l p) t -> p l t", p=128))
                for n8 in range(8):
                    wo_sb = ph4.tile([128, KC, 512], bf16, tag="wo", bufs=2,
                                     name="wo_sb")
                    nc.sync.dma_start(
                        out=wo_sb[:],
                        in_=Wo_t[n8].rearrange("p (k n) -> p k n", k=KC))
                    for b in range(2):
                        for th in range(2):
                            o_ps = psum.tile([128, 512], f32, tag="acc",
                                             bufs=3, name="o_ps")
                            for k in range(KC):
                                l = 4 * (k % 8) + (k // 8)
                                nc.tensor.matmul(
                                    o_ps[:],
                                    lhsT=att_sb[b][:, l,
                                                   128 * th:128 * (th + 1)],
                                    rhs=wo_sb[:, k, :],
                                    start=(k == 0), stop=(k == KC - 1))
                            o_sb = ph4.tile([128, 512], f32, tag="ostage",
                                            bufs=3, name="o_sb")
                            nc.vector.tensor_copy(out=o_sb[:], in_=o_ps[:])
                            r0 = 256 * b + 128 * th
                            nc.sync.dma_start(
                                out=out[r0:r0 + 128,
                                        512 * n8:512 * (n8 + 1)],
                                in_=o_sb[:])
    nc.finalize()
    return nc


def kernel(hidden_states, Wq, Wkv, Wo):
    global LAST_EXEC_NS
    bf = ml_dtypes.bfloat16
    hs = np.asarray(hidden_states, dtype=np.float32)
    Wq = np.asarray(Wq, dtype=np.float32)
    Wkv_np = np.asarray(Wkv, dtype=np.float32)
    Wo = np.asarray(Wo, dtype=np.float32)

    Wq_t = np.ascontiguousarray(
        Wq.reshape(KC, 128, 32, 128).transpose(2, 1, 0, 3)
        .reshape(32, 128, KC * 128)).astype(bf)
    Wo_t = np.ascontiguousarray(
        Wo.reshape(KC, 128, 8, 512).transpose(2, 1, 0, 3)
        .reshape(8, 128, KC * 512)).astype(bf)
    Wkv_t = np.ascontiguousarray(
        Wkv_np.reshape(KC, 128, 256).transpose(1, 0, 2)
        .reshape(128, KC * 256)).astype(bf)
    rel = _build_rel()
    slopes = _alibi_slopes(NH)

    in_maps = []
    for c in range(NC):
        blk = np.concatenate([hs[0, 256 * c:256 * (c + 1)],
                              hs[1, 256 * c:256 * (c + 1)]], axis=0)  # [512,H]
        hsT_c = np.ascontiguousarray(
            blk.T.reshape(KC, 128, TPC).transpose(1, 0, 2)
            .reshape(128, KC * TPC)).astype(bf)
        my_heads = [c + 8 * s for s in range(4)]
        slopes_c = np.ascontiguousarray(
            np.broadcast_to(slopes[my_heads][None, :], (128, 4)))
        in_maps.append({
            "hsT": hsT_c, "Wq_t": Wq_t, "Wkv": Wkv_t, "Wo_t": Wo_t,
            "rel": rel, "slopes": slopes_c,
        })

    if "nc" not in _CACHE:
        _CACHE["nc"] = _build_nc()
    nc = _CACHE["nc"]
    trace = bool(int(os.environ.get("BASS_KERNEL_TRACE", "0")))
    res = run_bass_kernel_spmd(nc, in_maps, core_ids=list(range(NC)),
                               trace=trace)
    LAST_EXEC_NS = res.exec_time_ns
    out_full = np.empty((B, S, H), np.float32)
    for c in range(NC):
        oc = res.results[c]["out"]
        out_full[0, 256 * c:256 * (c + 1)] = oc[0:256]
        out_full[1, 256 * c:256 * (c + 1)] = oc[256:512]
    return out_full


# revision 13
# speedup vs baseline: 1.1277x; 1.0104x over previous
"""MQA attention with ALiBi + causal mask on 8 TRN2 NeuronCores.

Problem: hidden_states [2,2048,4096] @ Wq -> 32 query heads of 128; single
KV head via Wkv; scores + ALiBi bias + causal mask; softmax; @ Wo.

Distribution (avoids the 64 MiB AllReduce of plain head-TP):
- Core c owns tokens [256c, 256(c+1)) of BOTH batches for all projections
  (output rows disjoint -> host concatenates). Attention is head-sharded
  round-robin (core c gets heads {c, c+8, c+16, c+24}) so every core's
  causal + ALiBi-cutoff workload is identical. The two shardings are
  bridged by small bf16 AllToAlls of qT / attnT plus AllGathers of the
  tiny single-head K/V. The qT AllToAlls are split per (batch, head-half)
  so attention starts while the q projection is still running, and
  batch-0's output projection is emission-interleaved with batch-1's
  attention so the PE-bound and DVE/ACT-bound work overlap.
- ALiBi distance cutoff: a kv chunk whose distance exceeds 45/slope
  contributes < 1e-13 of the softmax mass and is skipped. Per-slot
  (head-octile) uniform bounds keep the SPMD program identical per core.
- All matmuls in bf16 (rel-err budget 2e-2); softmax in f32 without
  max-subtraction (scores are O(10), bias <= 0 -> exp never overflows and
  the causal diagonal keeps denominators O(1)).

Softmax layout: scores are built transposed (scoresT[kpos, q]) so the
probs @ V matmul needs no transposes; kv chunks are processed in pairs
(one [128,512] bias-FMA + exp per pair); the denominator comes from
ones-stationary matmuls accumulated alongside and the per-q reciprocal is
broadcast across partitions with a K=1 outer-product matmul (f32r so it
runs at full speed without a cast). The ALiBi rel/mask tile depends only
on delta = j - 2g, so 16 distinct [128,256] tiles cover all (g, j).

Weights are pre-tiled on the host into SBUF partition-major layouts so
every weight DMA is fully contiguous.
"""
import math
import os

import numpy as np
import ml_dtypes

import concourse.bass as bass
from concourse import bacc
import concourse.mybir as mybir
from concourse.tile import TileContext
from concourse.bass_utils import run_bass_kernel_spmd

B, S, H, NH, HD = 2, 2048, 4096, 32, 128
NC = 8              # cores
TPC = 512           # tokens per core (256 per batch)
KC = H // 128       # 32 contraction chunks
GQ = 8              # 256-token q blocks per batch
SCALE = HD ** -0.5
# per-slot ALiBi reach (slot s = head octile): 45/slope maxed over octile
SLOT_D = [180.0, 1440.0, float("inf"), float("inf")]
USE_F32R_RB = False
bf16 = mybir.dt.bfloat16
f32 = mybir.dt.float32
f32r = mybir.dt.float32r
Exp = mybir.ActivationFunctionType.Exp
Copy = mybir.ActivationFunctionType.Copy
MULT = mybir.AluOpType.mult
ADD = mybir.AluOpType.add

_CACHE = {}
LAST_EXEC_NS = None


def _alibi_slopes(n_heads):
    closest_pow2 = 2 ** math.floor(math.log2(n_heads))
    base = 2.0 ** (-(2.0 ** -(math.log2(closest_pow2) - 3)))
    slopes = [base ** i for i in range(1, closest_pow2 + 1)]
    if closest_pow2 != n_heads:
        extra_base = 2.0 ** (-(2.0 ** -(math.log2(2 * closest_pow2) - 3)))
        n_extra = min(closest_pow2, n_heads - closest_pow2)
        slopes += [extra_base ** i for i in range(1, 2 * n_extra + 1, 2)]
    return np.asarray(slopes, dtype=np.float32)


def _j0(g, slot):
    d = SLOT_D[slot]
    if math.isinf(d):
        return 0
    return max(0, math.ceil((256 * g - 127 - d) / 128))


def _build_rel():
    # tile for delta = j - 2g at slice index (delta + 14): rel = 128*delta + p - f
    rel = np.empty((128, 16 * 256), np.float32)
    p = np.arange(128)[:, None]
    f = np.arange(256)[None, :]
    for idx in range(16):
        delta = idx - 14
        r = (128 * delta + p - f).astype(np.float32)
        r[128 * delta + p - f > 0] = -30000.0
        rel[:, 256 * idx:256 * (idx + 1)] = r
    return rel.astype(ml_dtypes.bfloat16)


def _build_nc():
    nc = bacc.Bacc(num_devices=NC)
    # host-pre-tiled layouts: every DMA below is contiguous in DRAM
    hsT = nc.declare_dram_parameter("hsT", [128, KC * TPC], bf16, isOutput=False)
    Wq_t = nc.declare_dram_parameter("Wq_t", [32, 128, KC * 128], bf16, isOutput=False)
    Wkv = nc.declare_dram_parameter("Wkv", [128, KC * 256], bf16, isOutput=False)
    Wo_t = nc.declare_dram_parameter("Wo_t", [8, 128, KC * 512], bf16, isOutput=False)
    rel = nc.declare_dram_parameter("rel", [128, 16 * 256], bf16, isOutput=False)
    slopes = nc.declare_dram_parameter("slopes", [128, 4], f32, isOutput=False)
    out = nc.declare_dram_parameter("out", [TPC, H], f32, isOutput=True)

    grp = [list(range(NC))]
    with TileContext(nc) as tc:
        with (
            tc.tile_pool(name="dram", bufs=1, space="DRAM") as dram,
            tc.tile_pool(name="const", bufs=1) as const,
            tc.tile_pool(name="psum", bufs=1, space="PSUM") as psum,
        ):
            kT_in = dram.tile([128, TPC], bf16)
            kT_ag = dram.tile([128 * NC, TPC], bf16, addr_space="Shared")
            v_in = dram.tile([TPC, 128], bf16)
            v_ag = dram.tile([TPC * NC, 128], bf16, addr_space="Shared")
            # q bounce buffers per (head-half hp, batch b)
            q_in = {(p_, b): dram.tile([H // 2, 256], bf16,
                                       name=f"q_in{p_}{b}")
                    for p_ in range(2) for b in range(2)}
            q_a2a = {(p_, b): dram.tile([H // 2, 256], bf16,
                                        name=f"q_a2a{p_}{b}")
                     for p_ in range(2) for b in range(2)}
            a_in = [dram.tile([H, 256], bf16, name=f"a_in{b}")
                    for b in range(2)]
            a_a2a = [dram.tile([H, 256], bf16, name=f"a_a2a{b}")
                     for b in range(2)]

            rel_sb = const.tile([128, 16 * 256], bf16)
            nc.sync.dma_start(out=rel_sb[:], in_=rel[:])
            slopes_sb = const.tile([128, 4], f32)
            nc.sync.dma_start(out=slopes_sb[:], in_=slopes[:])
            ones_col = const.tile([128, 1], bf16)
            nc.vector.memset(ones_col[:], 1.0)
            ones_row = const.tile([1, 128], f32r if USE_F32R_RB else bf16)
            nc.vector.memset(ones_row[:], 1.0)

            # ---------------- Phase 1: q/k/v projections -------------------
            with tc.tile_pool(name="ph1", bufs=1) as ph1:
                hsT_sb = ph1.tile([128, KC, TPC], bf16)
                nc.sync.dma_start(out=hsT_sb[:],
                                  in_=hsT.rearrange("p (k t) -> p k t", k=KC))
                Wkv_sb = ph1.tile([128, KC, 256], bf16)
                nc.sync.dma_start(out=Wkv_sb[:],
                                  in_=Wkv.rearrange("p (k c) -> p k c", k=KC))

                kT_ps = psum.tile([128, TPC], f32, tag="big", bufs=1)
                for k in range(KC):
                    nc.tensor.matmul(kT_ps[:], lhsT=Wkv_sb[:, k, 0:128],
                                     rhs=hsT_sb[:, k, :],
                                     start=(k == 0), stop=(k == KC - 1))
                kT_sb = ph1.tile([128, TPC], bf16)
                nc.vector.tensor_copy(out=kT_sb[:], in_=kT_ps[:])
                nc.sync.dma_start(out=kT_in[:], in_=kT_sb[:])

                for t4 in range(4):
                    v_ps = psum.tile([128, 128], f32, tag="big", bufs=1,
                                     name="v_ps")
                    for k in range(KC):
                        nc.tensor.matmul(
                            v_ps[:],
                            lhsT=hsT_sb[:, k, 128 * t4:128 * (t4 + 1)],
                            rhs=Wkv_sb[:, k, 128:256],
                            start=(k == 0), stop=(k == KC - 1))
                    v_sb = ph1.tile([128, 128], bf16, tag="v_sb", bufs=3,
                                    name="v_sb")
                    nc.vector.tensor_copy(out=v_sb[:], in_=v_ps[:])
                    nc.sync.dma_start(out=v_in[128 * t4:128 * (t4 + 1), :],
                                      in_=v_sb[:])

                # K/V AllGathers issue as soon as the tiny kv DMAs land,
                # overlapping the whole q projection below.
                nc.gpsimd.collective_compute(
                    "AllGather", mybir.AluOpType.bypass, replica_groups=grp,
                    ins=[kT_in[:]], outs=[kT_ag[:]])
                nc.gpsimd.collective_compute(
                    "AllGather", mybir.AluOpType.bypass, replica_groups=grp,
                    ins=[v_in[:]], outs=[v_ag[:]])

                # slots 0-1 heads first so their AllToAll can fire early
                dq_order = [d for d in range(32) if d // 8 < 2] + \
                           [d for d in range(32) if d // 8 >= 2]
                for dq in dq_order:
                    wq_sb = ph1.tile([128, KC, 128], bf16, tag="wq", bufs=3,
                                     name="wq_sb")
                    nc.sync.dma_start(
                        out=wq_sb[:],
                        in_=Wq_t[dq].rearrange("p (k m) -> p k m", k=KC))
                    q_ps = psum.tile([128, TPC], f32, tag="big", bufs=1,
                                     name="q_ps")
                    for k in range(KC):
                        nc.tensor.matmul(q_ps[:], lhsT=wq_sb[:, k, :],
                                         rhs=hsT_sb[:, k, :],
                                         start=(k == 0), stop=(k == KC - 1))
                    q_sb = ph1.tile([128, TPC], bf16, tag="qstage", bufs=3,
                                    name="q_sb")
                    nc.vector.tensor_scalar_mul(q_sb[:], q_ps[:], SCALE)
                    # head dq -> rank dq%8, slot dq//8 (round-robin heads)
                    hp_, sl_ = (dq // 8) // 2, (dq // 8) % 2
                    row = 256 * (dq % 8) + 128 * sl_
                    for b in range(2):
                        nc.sync.dma_start(
                            out=q_in[hp_, b][row:row + 128, :],
                            in_=q_sb[:, 256 * b:256 * (b + 1)])
                    if dq == dq_order[15]:  # slots 0-1 done -> ship half-A
                        for b in range(2):
                            nc.gpsimd.collective_compute(
                                "AllToAll", mybir.AluOpType.bypass,
                                replica_groups=grp,
                                ins=[q_in[0, b][:]], outs=[q_a2a[0, b][:]])
            for b in range(2):
                nc.gpsimd.collective_compute(
                    "AllToAll", mybir.AluOpType.bypass, replica_groups=grp,
                    ins=[q_in[1, b][:]], outs=[q_a2a[1, b][:]])

            # ---------------- Phases 3+4: attention & output projection ----
            # my slot-s head: global head = c + 8s; slope = slopes_sb[:, s]
            with (tc.tile_pool(name="attn", bufs=1) as attn,
                  tc.tile_pool(name="ph4", bufs=1) as ph4):
                kT_b, v_b, qT, aT = {}, {}, {}, {}
                for b in range(B):
                    t = attn.tile([128, 8, 256], bf16, name=f"kT_{b}")
                    nc.sync.dma_start(
                        out=t[:],
                        in_=kT_ag.rearrange("(r p) (b t) -> b p r t",
                                            p=128, b=2)[b])
                    kT_b[b] = t
                    t = attn.tile([128, 8, 2, 128], bf16, name=f"v_{b}")
                    for u in range(2):
                        nc.sync.dma_start(
                            out=t[:, :, u, :],
                            in_=v_ag.rearrange("(r b u p) d -> b p r u d",
                                               b=2, u=2, p=128)[b][:, :, u, :])
                    v_b[b] = t

                def kT_chunk(b, j):
                    return kT_b[b][:, j // 2, 128 * (j % 2):128 * (j % 2 + 1)]

                for b in range(B):
                    for s in range(4):
                        t = attn.tile([128, 8, 256], bf16, tag="qT", bufs=8,
                                      name=f"qT_{b}_{s}")
                        nc.sync.dma_start(
                            out=t[:],
                            in_=q_a2a[s // 2, b].rearrange(
                                "(j s p) t -> s p j t", s=2, p=128)[s % 2])
                        qT[b, s] = t
                        aT[b, s] = attn.tile([128, 8, 256], bf16, tag="aT",
                                             bufs=4, name=f"aT_{b}_{s}")

                def attn_unit(b, hp, g):
                    slots = (2 * hp, 2 * hp + 1)
                    nch = 2 * (g + 1)
                    j0 = [_j0(g, s) for s in slots]
                    at = [psum.tile([128, 256], f32, tag="at", bufs=2,
                                    name=f"at{hi}") for hi in range(2)]
                    den = [psum.tile([1, 256], f32, tag="den", bufs=2,
                                     name=f"den{hi}") for hi in range(2)]

                    def pv_den(hi, expp, base, ja, jb):
                        for ji, j in enumerate((ja, jb)):
                            if j is None:
                                continue
                            e_sl = expp[:, base + 256 * ji:base + 256 * (ji + 1)]
                            nc.tensor.matmul(
                                at[hi][:], lhsT=v_b[b][:, j // 2, j % 2, :],
                                rhs=e_sl, start=(j == j0[hi]),
                                stop=(j == nch - 1))
                            nc.tensor.matmul(
                                den[hi][:], lhsT=ones_col[:], rhs=e_sl,
                                start=(j == j0[hi]), stop=(j == nch - 1))

                    if j0[0] == j0[1]:
                        # identical chunk ranges: one wide exp per pair
                        js = list(range(j0[0], nch))
                        pairs = [(js[i], js[i + 1] if i + 1 < len(js)
                                  else None)
                                 for i in range(0, len(js), 2)]
                        for ja, jb in pairs:
                            w = 512 if jb is not None else 256
                            tmp = attn.tile([128, 1024], f32, tag="stt",
                                            bufs=2, name="tmp")
                            expp = attn.tile([128, 1024], bf16, tag="exp",
                                             bufs=4, name="expp")
                            for hi in range(2):
                                s2 = psum.tile([128, 512], f32, tag="s2",
                                               bufs=2, name="s2")
                                nc.tensor.matmul(
                                    s2[:, 0:256], lhsT=kT_chunk(b, ja),
                                    rhs=qT[b, slots[hi]][:, g, :],
                                    start=True, stop=True)
                                if jb is not None:
                                    nc.tensor.matmul(
                                        s2[:, 256:512], lhsT=kT_chunk(b, jb),
                                        rhs=qT[b, slots[hi]][:, g, :],
                                        start=True, stop=True)
                                d0 = 256 * (ja - 2 * g + 14)
                                nc.vector.scalar_tensor_tensor(
                                    out=tmp[:, w * hi:w * hi + w],
                                    in0=rel_sb[:, d0:d0 + w],
                                    scalar=slopes_sb[:, slots[hi]:
                                                     slots[hi] + 1],
                                    in1=s2[:, 0:w], op0=MULT, op1=ADD)
                            nc.scalar.activation(expp[:, 0:2 * w],
                                                 tmp[:, 0:2 * w], Exp)
                            for hi in range(2):
                                pv_den(hi, expp, w * hi, ja, jb)
                    else:
                        for hi in range(2):
                            sl = slots[hi]
                            js = list(range(j0[hi], nch))
                            pairs = [(js[i], js[i + 1] if i + 1 < len(js)
                                      else None)
                                     for i in range(0, len(js), 2)]
                            for ja, jb in pairs:
                                w = 512 if jb is not None else 256
                                s2 = psum.tile([128, 512], f32, tag="s2",
                                               bufs=2, name="s2")
                                nc.tensor.matmul(
                                    s2[:, 0:256], lhsT=kT_chunk(b, ja),
                                    rhs=qT[b, sl][:, g, :], start=True,
                                    stop=True)
                                if jb is not None:
                                    nc.tensor.matmul(
                                        s2[:, 256:512], lhsT=kT_chunk(b, jb),
                                        rhs=qT[b, sl][:, g, :], start=True,
                                        stop=True)
                                tmp = attn.tile([128, 512], f32, tag="stt2",
                                                bufs=2, name="tmp2")
                                expp = attn.tile([128, 512], bf16,
                                                 tag="exp2", bufs=4,
                                                 name="expp2")
                                d0 = 256 * (ja - 2 * g + 14)
                                nc.vector.scalar_tensor_tensor(
                                    out=tmp[:, 0:w],
                                    in0=rel_sb[:, d0:d0 + w],
                                    scalar=slopes_sb[:, sl:sl + 1],
                                    in1=s2[:, 0:w], op0=MULT, op1=ADD)
                                nc.scalar.activation(expp[:, 0:w],
                                                     tmp[:, 0:w], Exp)
                                pv_den(hi, expp, 0, ja, jb)
                    rec = attn.tile([1, 512], f32, tag="rec", bufs=2,
                                    name="rec")
                    for hi in range(2):
                        nc.vector.reciprocal_approx_fast(
                            out=rec[0:1, 256 * hi:256 * (hi + 1)],
                            in_=den[hi][:])
                    rb_ps = psum.tile([128, 512], f32, tag="rb", bufs=1,
                                      name="rb_ps")
                    if USE_F32R_RB:
                        nc.tensor.matmul(rb_ps[:], lhsT=ones_row[:],
                                         rhs=rec[:].bitcast(f32r),
                                         start=True, stop=True)
                    else:
                        rec_bf = attn.tile([1, 512], bf16, tag="rec_bf",
                                           bufs=2, name="rec_bf")
                        nc.vector.tensor_copy(out=rec_bf[:], in_=rec[:])
                        nc.tensor.matmul(rb_ps[:], lhsT=ones_row[:],
                                         rhs=rec_bf[:], start=True, stop=True)
                    rb_sb = attn.tile([128, 512], f32, tag="rbs", bufs=2,
                                      name="rb_sb")
                    nc.scalar.activation(rb_sb[:], rb_ps[:], Copy)
                    for hi in range(2):
                        nc.vector.tensor_tensor(
                            out=aT[b, slots[hi]][:, g, :], in0=at[hi][:],
                            in1=rb_sb[:, 256 * hi:256 * (hi + 1)], op=MULT)

                def ship_attnT(b):
                    for s in range(4):
                        nc.sync.dma_start(
                            out=a_in[b].rearrange("(j s p) t -> s p j t",
                                                  s=4, p=128)[s],
                            in_=aT[b, s][:])
                    nc.gpsimd.collective_compute(
                        "AllToAll", mybir.AluOpType.bypass,
                        replica_groups=grp,
                        ins=[a_in[b][:]], outs=[a_a2a[b][:]])

                att_sb = {}

                def load_att_sb(b):
                    att_sb[b] = ph4.tile([128, KC, 256], bf16,
                                         name=f"att_sb{b}")
                    nc.sync.dma_start(
                        out=att_sb[b][:],
                        in_=a_a2a[b].rearrange("(l p) t -> p l t", p=128))

                wo_tiles = {}

                def outproj_unit(b, n8, th):
                    if n8 not in wo_tiles:
                        w = ph4.tile([128, KC, 512], bf16, tag="wo", bufs=2,
                                     name="wo_sb")
                        nc.sync.dma_start(
                            out=w[:],
                            in_=Wo_t[n8].rearrange("p (k n) -> p k n", k=KC))
                        wo_tiles[n8] = w
                    wo_sb = wo_tiles[n8]
                    o_ps = psum.tile([128, 512], f32, tag="big", bufs=1,
                                     name="o_ps")
                    for k in range(KC):
                        l = 4 * (k % 8) + (k // 8)
                        nc.tensor.matmul(
                            o_ps[:],
                            lhsT=att_sb[b][:, l, 128 * th:128 * (th + 1)],
                            rhs=wo_sb[:, k, :],
                            start=(k == 0), stop=(k == KC - 1))
                    o_sb = ph4.tile([128, 512], f32, tag="ostage", bufs=2,
                                    name="o_sb")
                    nc.vector.tensor_copy(out=o_sb[:], in_=o_ps[:])
                    r0 = 256 * b + 128 * th
                    nc.sync.dma_start(
                        out=out[r0:r0 + 128, 512 * n8:512 * (n8 + 1)],
                        in_=o_sb[:])

                # batch 0 attention (hp0 can start during the q projection)
                for hp in range(2):
                    for g in range(GQ):
                        attn_unit(0, hp, g)
                ship_attnT(0)
                # batch 1 attention interleaved with batch 0 output proj
                load_att_sb(0)
                units_a = [(hp, g) for hp in range(2) for g in range(GQ)]
                units_o = [(n8, th) for n8 in range(8) for th in range(2)]
                for i in range(16):
                    attn_unit(1, *units_a[i])
                    outproj_unit(0, *units_o[i])
                ship_attnT(1)
                load_att_sb(1)
                wo_tiles.clear()
                for n8 in range(8):
                    for th in range(2):
                        outproj_unit(1, n8, th)
    nc.finalize()
    return nc


def kernel(hidden_states, Wq, Wkv, Wo):
    global LAST_EXEC_NS
    bf = ml_dtypes.bfloat16
    hs = np.asarray(hidden_states, dtype=np.float32)
    Wq = np.asarray(Wq, dtype=np.float32)
    Wkv_np = np.asarray(Wkv, dtype=np.float32)
    Wo = np.asarray(Wo, dtype=np.float32)

    Wq_t = np.ascontiguousarray(
        Wq.reshape(KC, 128, 32, 128).transpose(2, 1, 0, 3)
        .reshape(32, 128, KC * 128)).astype(bf)
    Wo_t = np.ascontiguousarray(
        Wo.reshape(KC, 128, 8, 512).transpose(2, 1, 0, 3)
        .reshape(8, 128, KC * 512)).astype(bf)
    Wkv_t = np.ascontiguousarray(
        Wkv_np.reshape(KC, 128, 256).transpose(1, 0, 2)
        .reshape(128, KC * 256)).astype(bf)
    rel = _build_rel()
    slopes = _alibi_slopes(NH)

    in_maps = []
    for c in range(NC):
        blk = np.concatenate([hs[0, 256 * c:256 * (c + 1)],
                              hs[1, 256 * c:256 * (c + 1)]], axis=0)
        hsT_c = np.ascontiguousarray(
            blk.T.reshape(KC, 128, TPC).transpose(1, 0, 2)
            .reshape(128, KC * TPC)).astype(bf)
        my_heads = [c + 8 * s for s in range(4)]
        slopes_c = np.ascontiguousarray(
            np.broadcast_to(slopes[my_heads][None, :], (128, 4)))
        in_maps.append({
            "hsT": hsT_c, "Wq_t": Wq_t, "Wkv": Wkv_t, "Wo_t": Wo_t,
            "rel": rel, "slopes": slopes_c,
        })

    if "nc" not in _CACHE:
        _CACHE["nc"] = _build_nc()
    nc = _CACHE["nc"]
    trace = bool(int(os.environ.get("BASS_KERNEL_TRACE", "0")))
    res = run_bass_kernel_spmd(nc, in_maps, core_ids=list(range(NC)),
                               trace=trace)
    LAST_EXEC_NS = res.exec_time_ns
    out_full = np.empty((B, S, H), np.float32)
    for c in range(NC):
        oc = res.results[c]["out"]
        out_full[0, 256 * c:256 * (c + 1)] = oc[0:256]
        out_full[1, 256 * c:256 * (c + 1)] = oc[256:512]
    return out_full
